# revision 38
# baseline (speedup 1.0000x reference)
"""Trainium2 Bass kernel for a 2-layer GCN + global mean pool + MLP head.

Model (reference semantics):
    h1  = relu(GCNConv(x, W1, b1))          # symmetric-normalized A+I aggregation
    h2  = GCNConv(h1, W2, b2)
    g   = global_mean_pool(h2, batch)        # 512 graphs
    out = (relu(g @ W3 + b3) @ W4 + b4)      # [512]

Distribution: nodes (and their in-edges) sharded contiguously across the 8
NeuronCores; the per-layer scaled feature table hs = (x @ W) * dinv is
all-gathered so every core can gather messages for its local destination
nodes; edge aggregation is a one-hot (selection-matrix) matmul accumulated in
PSUM per 128-destination tile; pooled sums/counts are all-reduced.

The edge gather uses dma_gather (SWDGE) against a bf16 row-padded feature
table (256B rows) in HBM, banked by 32768 rows to fit int16 indices.

Degree normalization (dinv = rsqrt(deg)) depends only on edge_index, so it is
precomputed on the host and uploaded; the per-node bias is folded into the
PSUM edge aggregation as a rank-1 bf16 matmul outer(sqrt(deg), b) so the
whole conv epilogue is one DVE add (self term) + one scalar-engine
Relu/Copy with per-partition dinv scale.
"""

import numpy as np

P = 128
DIM = 64


class CFG:
    def __init__(self, n=100000, e=1600000, g=512, cores=8, maxch_call=32):
        self.N = n
        self.E = e
        self.G = g
        self.R = cores
        self.PC = -(-n // cores)            # nodes per core (pre-pad)
        self.PC = -(-self.PC // P) * P      # pad to 128
        self.T = self.PC // P               # dst tiles per core
        self.NP = self.PC * cores           # padded global nodes
        self.NB = 4                         # banks = tile quarters
        # quarter boundaries in local tiles; bank k spans tiles [qb[k], qb[k+1])
        # skewed: small first quarter so the first sub-AllGather (which gates
        # the first gather call) fires early in the feature pass
        self.qb = [0, self.T // 7, self.T * 3 // 7, self.T * 5 // 7, self.T]
        self.qrows = [(self.qb[k + 1] - self.qb[k]) * P
                      for k in range(self.NB)]       # per-core rows in bank k
        self.subrows = [r * cores for r in self.qrows]   # total rows in bank k
        assert max(self.subrows) <= 32767, self.subrows
        self.MAXCH = maxch_call             # max chunks per gather call
        self.skip_gather = False
        self.skip_edge_mm = False
        self.nqueues = 4
        self.skip_oh = False
        self.skip_ag = False
        self.skip_tail = False
        self.repeat = 1
        self.msg_bufs = 6
        self.zero_idx = False
        self.late_ag = False
        self.xt_bufs = 2
        self.oh_bufs = 8
        self.sp_all = False


FULL = CFG()


# ---------------------------------------------------- host preprocessing ---

def _prep(edge_index, batch, cfg):
    """Partition/pad edges; build per-core device arrays + shared schedule."""
    c = cfg
    src = np.asarray(edge_index[0], dtype=np.int64)
    dst = np.asarray(edge_index[1], dtype=np.int64)
    batch = np.asarray(batch, dtype=np.int64)

    core = dst // c.PC
    tloc = (dst % c.PC) // P
    slot = dst % P
    # bank = tile-quarter of the source node; row = core-major within bank
    qb = np.asarray(c.qb)
    qrows = np.asarray(c.qrows)
    csrc = src // c.PC
    lt = (src % c.PC) // P
    sslot = src % P
    bank = np.searchsorted(qb, lt, side="right") - 1
    ib = (csrc * qrows[bank] + (lt - qb[bank]) * P + sslot).astype(np.int64)

    key = ((core * c.T + tloc) * c.NB + bank)
    order = np.lexsort((ib, key))
    key_s = key[order]
    ib_s = ib[order].astype(np.int16)
    slot_s = slot[order].astype(np.float32)

    nkey = c.R * c.T * c.NB
    cnts = np.bincount(key_s, minlength=nkey)
    counts = cnts.reshape(c.R, c.T, c.NB)
    starts_flat = np.concatenate([[0], np.cumsum(cnts)])

    C_tb = -(-counts.max(axis=0) // P)          # [T, NB]
    for t in range(c.T):
        if C_tb[t].sum() == 0:
            C_tb[t, 0] = 1

    groups = []
    t0 = 0
    while t0 < c.T:
        t1 = t0
        while t1 < c.T:
            nch = C_tb[t0:t1 + 1].sum(axis=0).max()
            if nch > c.MAXCH and t1 > t0:
                break
            t1 += 1
        groups.append((t0, t1))
        t0 = t1

    chunk_of = np.zeros((c.T, c.NB), dtype=np.int64)
    calls = []       # (bank, t0, t1, chunk0, nch, idx_col0)
    CH = 0
    icol = 0
    for (t0, t1) in groups:
        for b in range(c.NB):
            ch0 = CH
            for t in range(t0, t1):
                chunk_of[t, b] = CH
                CH += int(C_tb[t, b])
            nch = CH - ch0
            if nch:
                calls.append((b, t0, t1, ch0, int(nch), icol))
                icol += nch * P // 16
    sched = dict(C_tb=C_tb, chunk_of=chunk_of, groups=groups, calls=calls,
                 CH=int(CH), ICOLS=int(icol))

    # degree (in-degree incl self-loop) -> dinv, sqrt(deg); host-only
    deg_all = np.bincount(dst, minlength=c.N).astype(np.float64) + 1.0
    dinv_all = (1.0 / np.sqrt(deg_all)).astype(np.float32)
    sdeg_all = np.sqrt(deg_all).astype(np.float32)
    # padded global dinv, tile-major [128, R*T] (identical on every core)
    dinvP = np.ones(c.NP, dtype=np.float32)
    for r in range(c.R):
        n0 = r * c.PC
        nr = max(0, min(c.N - n0, c.PC))
        if nr:
            dinvP[r * c.PC:r * c.PC + nr] = dinv_all[n0:n0 + nr]
    dinvG = dinvP.reshape(c.R * c.T, P).T.copy()          # [128, TG]

    per_core = []
    for r in range(c.R):
        idxw = np.zeros((P, icol), dtype=np.int16)
        dstl = np.full((P, CH), -1.0, dtype=np.float32)
        for (b, t0, t1, ch0, nch, col0) in calls:
            # pad slots gather spread-out (harmless) rows: a constant pad
            # row serializes on one HBM bank and stalls the whole queue
            li = ((np.arange(nch * P, dtype=np.int64) * 97 + b * 1009)
                  % c.subrows[b]).astype(np.int16)
            for t in range(t0, t1):
                k = (r * c.T + t) * c.NB + b
                s0, s1 = starts_flat[k], starts_flat[k + 1]
                n = int(s1 - s0)
                if n == 0:
                    continue
                o = int(chunk_of[t, b] - ch0) * P
                li[o:o + n] = ib_s[s0:s1]
                cpos = int(chunk_of[t, b])
                ii = np.arange(n)
                dstl[ii % P, cpos + ii // P] = slot_s[s0:s1]
            w = li.reshape(-1, 16).T                      # [16, ncol]
            idxw[:, col0:col0 + nch * P // 16] = np.tile(w, (8, 1))
        n0 = r * c.PC
        nreal = max(0, min(c.N - n0, c.PC))
        bat = np.full(c.PC, -1.0, dtype=np.float32)
        dv = np.ones(c.PC, dtype=np.float32)
        sd = np.ones(c.PC, dtype=np.float32)
        if nreal > 0:
            bat[:nreal] = batch[n0:n0 + nreal].astype(np.float32)
            dv[:nreal] = dinv_all[n0:n0 + nreal]
            sd[:nreal] = sdeg_all[n0:n0 + nreal]
        batchF = bat.reshape(c.T, P).T.copy()             # [128, T]
        dinvF = dv.reshape(c.T, P).T.copy()               # [128, T]
        sdegF = sd.reshape(1, c.PC).copy()                # [1, PC]
        if nreal > 0:
            g0 = int(batch[n0])
            ghi = int(batch[min(n0 + nreal, c.N) - 1])
            assert ghi - g0 < P, (r, g0, ghi)
        else:
            g0 = c.G - 1
        per_core.append(dict(idxw=idxw, dstl=dstl, batchF=batchF, g0=g0,
                             dinvF=dinvF, sdegF=sdegF, dinvG=dinvG))
    return sched, per_core


# ------------------------------------------------------- program builder ---

def build_program(cfg, sched):
    import concourse.bass as bass
    import concourse.bacc as bacc
    import concourse.mybir as mybir
    import concourse.tile as tile
    from concourse.tile import add_dep_helper

    c = cfg
    dt = mybir.dt
    f32 = dt.float32
    edt = dt.bfloat16
    ROWE = P                             # table row elements (256B rows)
    C_tb, chunk_of, calls = sched["C_tb"], sched["chunk_of"], sched["calls"]
    CH, ICOLS = sched["CH"], sched["ICOLS"]
    T, NB = c.T, c.NB
    ACT = mybir.ActivationFunctionType

    def tile_chunks(t):
        return [(b, k) for b in range(NB) for k in range(int(C_tb[t, b]))]

    def call_chunk0(t, b):
        for (bb, tt0, tt1, c0, nn, _c) in calls:
            if bb == b and tt0 <= t < tt1:
                return c0
        raise AssertionError((t, b))

    nc = bacc.Bacc("TRN2", target_bir_lowering=False, debug=False,
                   num_devices=c.R, num_swdge_queues=c.nqueues)

    TG = c.T * c.R                       # global padded tiles

    # ---- I/O ----
    xT_p = nc.declare_dram_parameter("xT", [DIM, c.PC], f32, isOutput=False)
    W_p = [nc.declare_dram_parameter(f"W{i+1}", [DIM, DIM if i < 3 else 1],
                                     f32, isOutput=False) for i in range(4)]
    b_p = [nc.declare_dram_parameter(f"b{i+1}", [1, DIM if i < 3 else 1],
                                     f32, isOutput=False) for i in range(4)]
    bbf_p = [nc.declare_dram_parameter(f"bb{i+1}", [1, DIM], edt,
                                       isOutput=False) for i in range(2)]
    iotaM_p = nc.declare_dram_parameter("iotaM", [P, P], f32, isOutput=False)
    id128_p = nc.declare_dram_parameter("id128", [P, P], f32, isOutput=False)
    ones1_p = nc.declare_dram_parameter("ones1", [1, P], f32, isOutput=False)
    idxw_p = nc.declare_dram_parameter("idxw", [P, ICOLS], dt.int16,
                                       isOutput=False)
    dstl_p = nc.declare_dram_parameter("dstl", [P, CH], f32, isOutput=False)
    batchF_p = nc.declare_dram_parameter("batchF", [P, T], f32, isOutput=False)
    dinvF_p = nc.declare_dram_parameter("dinvF", [P, T], f32, isOutput=False)
    sdegF_p = nc.declare_dram_parameter("sdegF", [1, c.PC], edt,
                                        isOutput=False)
    g0f_p = nc.declare_dram_parameter("g0f", [1, 1], f32, isOutput=False)
    g0i_p = nc.declare_dram_parameter("g0i", [1, 1], dt.uint32, isOutput=False)
    pred_p = nc.declare_dram_parameter("pred", [c.G, 1], f32, isOutput=True)

    # ---- internal DRAM (per repeat for clean timing isolation) ----
    GTOT = -(-(c.G + P) // P) * P
    hsl_a, tab1_a, tab2_a, pool_loc_a, pool_red_a = [], [], [], [], []
    for r_ in range(c.repeat):
        hsl_a.append([nc.dram_tensor(f"hs{i}_local_{r_}", [c.PC, ROWE], edt)
                      for i in (1, 2)])
        tab1_a.append([nc.dram_tensor(f"hs1_tab{k}_{r_}",
                                      [c.subrows[k], ROWE], edt,
                                      addr_space="Shared")
                       for k in range(c.NB)])
        tab2_a.append([nc.dram_tensor(f"hs2_tab{k}_{r_}",
                                      [c.subrows[k], ROWE], edt,
                                      addr_space="Shared")
                       for k in range(c.NB)])
        pool_loc_a.append(nc.dram_tensor(f"pool_local_{r_}",
                                         [GTOT, DIM + 1], f32))
        pool_red_a.append(nc.dram_tensor(f"pool_red_{r_}", [GTOT, DIM + 1],
                                         f32, addr_space="Shared"))

    rg = [list(range(c.R))]

    with tile.TileContext(nc) as tc:
        with (
            tc.tile_pool(name="const", bufs=1) as cpool,
            tc.tile_pool(name="msg", bufs=c.msg_bufs) as mpool,
            tc.tile_pool(name="oh", bufs=c.oh_bufs) as ohpool,
            tc.tile_pool(name="work", bufs=3) as wpool,
            tc.tile_pool(name="ps_agg", bufs=2, space="PSUM") as pagg,
            tc.tile_pool(name="ps_h", bufs=2, space="PSUM") as ph,
            tc.tile_pool(name="ps_tr", bufs=2, space="PSUM") as ptr,
            tc.tile_pool(name="ps_pool", bufs=1, space="PSUM") as ppool,
        ):

            def load(pool, ap, shape, dtype=f32, name=None):
                t = pool.tile(shape, dtype, tag=name)
                nc.sync.dma_start(out=t[:], in_=ap)
                return t

            iotaM = load(cpool, iotaM_p[:, :], [P, P], name="iotaM")
            id128 = load(cpool, id128_p[:, :], [P, P], name="id128")
            ones1 = load(cpool, ones1_p[:, :], [1, P], name="ones1")
            Ws = [load(cpool, W_p[i][:, :], [DIM, DIM if i < 3 else 1],
                       name=f"W{i}") for i in range(4)]
            bs = [load(cpool, b_p[i][:, :], [1, DIM if i < 3 else 1],
                       name=f"b{i}") for i in range(4)]
            bbf = [load(cpool, bbf_p[i][:, :], [1, DIM], edt, name=f"bbf{i}")
                   for i in range(2)]
            idxw = load(cpool, idxw_p[:, :], [P, ICOLS], dt.int16, name="idxw")
            batchF = load(cpool, batchF_p[:, :], [P, T], name="batchF")
            dinv = load(cpool, dinvF_p[:, :], [P, T], name="dinvF")
            sdeg = load(cpool, sdegF_p[:, :], [1, c.PC], edt, name="sdegF")
            g0f = load(cpool, g0f_p[:, :], [1, 1], name="g0f")

            iotaE = cpool.tile([P, P], edt, tag="iotaE")
            nc.vector.tensor_copy(out=iotaE[:], in_=iotaM[:])
            dstl = load(cpool, dstl_p[:, :], [P, CH], name="dstl")

            # bias broadcast mats via PE outer product: ones1.T @ b
            bB = []
            for i in range(2, 4):
                w = DIM if i < 3 else 1
                pb = ptr.tile([P, w], f32, tag="tr")
                nc.tensor.matmul(out=pb[:], lhsT=ones1[:], rhs=bs[i][:],
                                 start=True, stop=True)
                sb = cpool.tile([P, w], f32, tag=f"bB{i}")
                nc.scalar.copy(out=sb[:], in_=pb[:])
                bB.append(sb)
            pg = ptr.tile([P, 1], f32, tag="tr")
            nc.tensor.matmul(out=pg[:], lhsT=ones1[:], rhs=g0f[:],
                             start=True, stop=True)
            g0B = cpool.tile([P, 1], f32, tag="g0B")
            nc.scalar.copy(out=g0B[:], in_=pg[:])

            # persistent SBUF feature tables (f32, this core's nodes)
            hs_sb = [cpool.tile([P, T * DIM], f32, tag=f"hs_sb{i}",
                                name=f"hs_sb{i}") for i in range(2)]

            # rotating bf16 row-padded staging tiles (pad cols pre-zeroed)
            hsb_bufs = [cpool.tile([P, ROWE], edt, tag=f"hsb{i}",
                                   name=f"hsb{i}") for i in range(4)]
            for hb in hsb_bufs:
                nc.vector.memset(hb[:, DIM:ROWE], 0.0)

            # pool staging tiles with constant ones column
            pt_bufs = [cpool.tile([P, DIM + 1], edt, tag=f"pt{i}",
                                  name=f"pt{i}") for i in range(3)]
            for pb_ in pt_bufs:
                nc.vector.memset(pb_[:, DIM:DIM + 1], 1.0)

            oh_const = cpool.tile([P, P], edt, tag="ohc")
            nc.vector.memset(oh_const[:], 0.0)

            def onehot_for(chunk):
                if c.skip_oh:
                    return oh_const
                oh = ohpool.tile([P, P], edt, tag="oh")
                nc.vector.tensor_scalar(
                    out=oh[:], in0=iotaE[:],
                    scalar1=dstl[:, chunk:chunk + 1],
                    scalar2=None, op0=mybir.AluOpType.is_equal)
                return oh

            for _rep in range(c.repeat):
                hsl_r = hsl_a[_rep]
                tab1 = tab1_a[_rep]
                tab2 = tab2_a[_rep]
                pool_loc_r = pool_loc_a[_rep]
                pool_red_r = pool_red_a[_rep]

                ag_insts = [[None] * c.NB, [None] * c.NB]

                # ---------- hs writer: per-tile rows + quartered sub-AGs ----
                def emit_hs(t, hp, layer_i):
                    """hs = hp * dinv[t]; f32 in SBUF + bf16 row to HBM;
                    fire the sub-AllGather once a tile quarter completes."""
                    nc.scalar.mul(out=hs_sb[layer_i][:, t * DIM:(t + 1) * DIM],
                                  in_=hp[:], mul=dinv[:, t:t + 1])
                    hsb = hsb_bufs[t % len(hsb_bufs)]
                    nc.scalar.mul(out=hsb[:, 0:DIM], in_=hp[:],
                                  mul=dinv[:, t:t + 1])
                    nc.sync.dma_start(
                        out=hsl_r[layer_i][t * P:(t + 1) * P, :], in_=hsb[:])
                    tabs = tab1 if layer_i == 0 else tab2
                    for k in range(c.NB):
                        fire_at = T - 1 if c.late_ag else c.qb[k + 1] - 1
                        if t == fire_at and not c.skip_ag:
                            ag_insts[layer_i][k] = nc.gpsimd.collective_compute(
                                "AllGather", mybir.AluOpType.bypass,
                                replica_groups=rg,
                                ins=[hsl_r[layer_i][c.qb[k] * P:
                                                    c.qb[k + 1] * P, :]],
                                outs=[tabs[k][:, :]])

                # ---------- layer-1 features: h1 = x @ W1 ----------
                with tc.tile_pool(name="xTp", bufs=c.xt_bufs) as xpool:
                    for k in range(c.NB):
                        tq0, tq1 = c.qb[k], c.qb[k + 1]
                        xTq = xpool.tile([DIM, (tq1 - tq0) * P], f32,
                                         tag="xT", name="xTq")
                        nc.sync.dma_start(out=xTq[:],
                                          in_=xT_p[:, tq0 * P:tq1 * P])
                        for t in range(tq0, tq1):
                            hp = ph.tile([P, DIM], f32, tag="h")
                            nc.tensor.matmul(
                                out=hp[:],
                                lhsT=xTq[:, (t - tq0) * P:(t - tq0 + 1) * P],
                                rhs=Ws[0][:], start=True, stop=True)
                            emit_hs(t, hp, 0)

                # ---------- edge-aggregation layer ----------
                def run_layer(tabs, layer_i, bbf_t, ag_insts, emit_tile_out):
                    gat_of = {}
                    for ci, (b, t0, t1, ch0, nchv, col0) in enumerate(calls):
                        m = mpool.tile([P, nchv, ROWE], edt, tag="msg")
                        if not c.skip_gather:
                            g = nc.gpsimd.dma_gather(
                                m[:], tabs[b][0:c.subrows[b], :],
                                idxw[:, col0:col0 + nchv * P // 16], nchv * P,
                                nchv * P, ROWE,
                                single_packet=(c.sp_all or nchv * P <= 1024),
                                queue_num=ci % c.nqueues)
                            if ag_insts is not None and ag_insts[b] is not None:
                                add_dep_helper(g.ins, ag_insts[b].ins)
                        else:
                            nc.vector.memset(m[:, 0, :], 0.0)
                        for t in range(t0, t1):
                            gat_of[(t, b)] = m
                    for t in range(T):
                        chunks = tile_chunks(t)
                        if c.skip_edge_mm:
                            chunks = chunks[:1]
                        ap = pagg.tile([P, DIM], f32, tag="agg")
                        # rank-1 bias term: outer(sqrt(deg), b) in bf16
                        nc.tensor.matmul(
                            out=ap[:], lhsT=sdeg[:, t * P:(t + 1) * P],
                            rhs=bbf_t[:], start=True, stop=False)
                        for j, (b, k) in enumerate(chunks):
                            oh = onehot_for(int(chunk_of[t, b]) + k)
                            m = gat_of[(t, b)]
                            kk = int(chunk_of[t, b]) - call_chunk0(t, b) + k
                            nc.tensor.matmul(
                                out=ap[:], lhsT=oh[:], rhs=m[:, kk, 0:DIM],
                                start=False, stop=(j == len(chunks) - 1))
                        z = wpool.tile([P, DIM], f32, tag="z")
                        nc.vector.tensor_tensor(
                            out=z[:], in0=ap[:],
                            in1=hs_sb[layer_i][:, t * DIM:(t + 1) * DIM],
                            op=mybir.AluOpType.add)
                        emit_tile_out(t, z)

                # layer-1 tile epilogue: relu*dinv -> transpose -> h2 -> hs2
                def l1_out(t, z):
                    o1 = wpool.tile([P, DIM], f32, tag="o1")
                    nc.scalar.activation(out=o1[:], in_=z[:], func=ACT.Relu,
                                         scale=dinv[:, t:t + 1])
                    tp = ptr.tile([DIM, P], f32, tag="tr")
                    nc.tensor.transpose(out=tp[:], in_=o1[:], identity=id128[:])
                    o1T = wpool.tile([DIM, P], f32, tag="o1Ts")
                    nc.scalar.copy(out=o1T[:], in_=tp[:])
                    hp = ph.tile([P, DIM], f32, tag="h")
                    nc.tensor.matmul(out=hp[:], lhsT=o1T[:], rhs=Ws[1][:],
                                     start=True, stop=True)
                    emit_hs(t, hp, 1)

                run_layer(tab1, 0, bbf[0], ag_insts[0], l1_out)

                psum_pool = ppool.tile([P, DIM + 1], f32, tag="pool")

                def l2_out(t, z):
                    pt = pt_bufs[t % len(pt_bufs)]
                    nc.scalar.mul(out=pt[:, 0:DIM], in_=z[:],
                                  mul=dinv[:, t:t + 1])
                    og = ohpool.tile([P, P], edt, tag="ohg")
                    nc.vector.tensor_scalar(
                        out=og[:], in0=iotaM[:], scalar1=g0B[:, 0:1],
                        scalar2=batchF[:, t:t + 1], op0=mybir.AluOpType.add,
                        op1=mybir.AluOpType.is_equal)
                    nc.tensor.matmul(out=psum_pool[:], lhsT=og[:], rhs=pt[:],
                                     start=(t == 0), stop=(t == T - 1))

                run_layer(tab2, 1, bbf[1], ag_insts[1], l2_out)

                # ---------- pool finalize + AllReduce ----------
                poolsb = wpool.tile([P, DIM + 1], f32, tag="poolsb")
                nc.scalar.copy(out=poolsb[:], in_=psum_pool[:])
                if c.skip_tail:
                    nc.sync.dma_start(out=pred_p[0:P, :], in_=poolsb[:, 0:1])
                else:
                    zt = wpool.tile([P, DIM + 1], f32, tag="zt")
                    nc.vector.memset(zt[:], 0.0)
                    zdmas = []
                    for q in range(GTOT // P):
                        zdmas.append(nc.sync.dma_start(
                            out=pool_loc_r[q * P:(q + 1) * P, :], in_=zt[:]))
                    g0reg = nc.sync.alloc_register(f"g0reg{_rep}")
                    nc.sync.reg_load(g0reg, g0i_p[0:1, 0:1])
                    g0val = nc.sync.snap(g0reg, donate=True, min_val=0,
                                         max_val=GTOT - P)
                    wdma = nc.sync.dma_start(
                        out=pool_loc_r[bass.ds(g0val, P), :], in_=poolsb[:])
                    for zd in zdmas:
                        add_dep_helper(wdma.ins, zd.ins)
                    ar = nc.gpsimd.collective_compute(
                        "AllReduce", mybir.AluOpType.add, replica_groups=rg,
                        ins=[pool_loc_r[:, :]], outs=[pool_red_r[:, :]])
                    add_dep_helper(ar.ins, wdma.ins)

                    # ---------- head ----------
                    for q in range(-(-c.G // P)):
                        gq = min(P, c.G - q * P)
                        S = wpool.tile([P, DIM + 1], f32, tag="S")
                        d = nc.sync.dma_start(out=S[:],
                                              in_=pool_red_r[q * P:q * P + P, :])
                        add_dep_helper(d.ins, ar.ins)
                        cm = wpool.tile([P, 1], f32, tag="cm")
                        nc.vector.tensor_scalar(out=cm[:], in0=S[:, DIM:DIM + 1],
                                                scalar1=1.0, scalar2=None,
                                                op0=mybir.AluOpType.max)
                        ci = wpool.tile([P, 1], f32, tag="ci")
                        nc.vector.reciprocal(out=ci[:], in_=cm[:])
                        gt = wpool.tile([P, DIM], f32, tag="gt")
                        nc.vector.tensor_scalar(out=gt[:], in0=S[:, 0:DIM],
                                                scalar1=ci[:, 0:1], scalar2=None,
                                                op0=mybir.AluOpType.mult)
                        tp = ptr.tile([DIM, P], f32, tag="tr")
                        nc.tensor.transpose(out=tp[:], in_=gt[:], identity=id128[:])
                        gT = wpool.tile([DIM, P], f32, tag="gT")
                        nc.scalar.copy(out=gT[:], in_=tp[:])
                        zp = ph.tile([P, DIM], f32, tag="h")
                        nc.tensor.matmul(out=zp[:], lhsT=gT[:], rhs=Ws[2][:],
                                         start=True, stop=True)
                        zz = wpool.tile([P, DIM], f32, tag="zz")
                        nc.vector.tensor_tensor(out=zz[:], in0=zp[:], in1=bB[0][:],
                                                op=mybir.AluOpType.add)
                        nc.vector.tensor_scalar(out=zz[:], in0=zz[:], scalar1=0.0,
                                                scalar2=None, op0=mybir.AluOpType.max)
                        tp2 = ptr.tile([DIM, P], f32, tag="tr")
                        nc.tensor.transpose(out=tp2[:], in_=zz[:], identity=id128[:])
                        zT = wpool.tile([DIM, P], f32, tag="zT")
                        nc.scalar.copy(out=zT[:], in_=tp2[:])
                        pp = ptr.tile([P, 1], f32, tag="tr")
                        nc.tensor.matmul(out=pp[:], lhsT=zT[:], rhs=Ws[3][:],
                                         start=True, stop=True)
                        pr = wpool.tile([P, 1], f32, tag="pr")
                        nc.vector.tensor_tensor(out=pr[:], in0=pp[:], in1=bB[1][:],
                                                op=mybir.AluOpType.add)
                        nc.sync.dma_start(out=pred_p[q * P:q * P + gq, :],
                                          in_=pr[:gq, :])
    nc.compile()
    return nc


# --------------------------------------------------------------- runner ---

def _make_in_maps(x, W1, b1, W2, b2, W3, b3, W4, b4, cfg, per_core):
    import ml_dtypes
    c = cfg
    iotaM = np.tile(np.arange(P, dtype=np.float32)[None, :], (P, 1))
    id128 = np.eye(P, dtype=np.float32)
    ones1 = np.ones((1, P), dtype=np.float32)
    bf16 = ml_dtypes.bfloat16
    xf = np.zeros((c.NP, DIM), dtype=np.float32)
    for r in range(c.R):
        n0 = r * c.PC
        nreal = max(0, min(c.N - n0, c.PC))
        if nreal:
            xf[r * c.PC:r * c.PC + nreal] = np.asarray(
                x[n0:n0 + nreal], dtype=np.float32)
    xTf = np.ascontiguousarray(xf.T)
    maps = []
    for r in range(c.R):
        pc = per_core[r]
        maps.append({
            "xT": np.ascontiguousarray(xf[r * c.PC:(r + 1) * c.PC].T),
            "W1": np.asarray(W1, np.float32),
            "W2": np.asarray(W2, np.float32),
            "W3": np.asarray(W3, np.float32),
            "W4": np.asarray(W4, np.float32).reshape(DIM, 1),
            "b1": np.asarray(b1, np.float32).reshape(1, DIM),
            "b2": np.asarray(b2, np.float32).reshape(1, DIM),
            "b3": np.asarray(b3, np.float32).reshape(1, DIM),
            "b4": np.asarray(b4, np.float32).reshape(1, 1),
            "bb1": np.asarray(b1, np.float32).reshape(1, DIM).astype(bf16),
            "bb2": np.asarray(b2, np.float32).reshape(1, DIM).astype(bf16),
            "iotaM": iotaM, "id128": id128, "ones1": ones1,
            "idxw": (np.zeros_like(pc["idxw"]) if c.zero_idx
                     else pc["idxw"]),
            "dstl": pc["dstl"], "batchF": pc["batchF"],
            "dinvF": pc["dinvF"],
            "sdegF": pc["sdegF"].astype(bf16),
            "g0f": np.array([[float(pc["g0"])]], dtype=np.float32),
            "g0i": np.array([[pc["g0"]]], dtype=np.uint32),
        })
    return maps


def kernel(x, edge_index, batch, W1, b1, W2, b2, W3, b3, W4, b4,
           cfg=None, run=None):
    import sys
    if "/opt/trn_rl_repo" not in sys.path:
        sys.path.insert(0, "/opt/trn_rl_repo")
    cfg = cfg or FULL
    x = np.asarray(x)
    edge_index = np.asarray(edge_index)
    batch = np.asarray(batch)
    sched, per_core = _prep(edge_index, batch, cfg)
    nc = build_program(cfg, sched)
    maps = _make_in_maps(x, W1, b1, W2, b2, W3, b3, W4, b4, cfg, per_core)
    if run is not None:                 # custom runner (e.g. simulator)
        return run(nc, maps)
    from concourse.bass_utils import run_bass_kernel_spmd
    res = run_bass_kernel_spmd(nc, maps, list(range(cfg.R)))
    return np.asarray(res.results[0]["pred"]).reshape(-1).astype(np.float32)


# revision 43
# speedup vs baseline: 1.0026x; 1.0026x over previous
"""Trainium2 Bass kernel for a 2-layer GCN + global mean pool + MLP head.

Model (reference semantics):
    h1  = relu(GCNConv(x, W1, b1))          # symmetric-normalized A+I aggregation
    h2  = GCNConv(h1, W2, b2)
    g   = global_mean_pool(h2, batch)        # 512 graphs
    out = (relu(g @ W3 + b3) @ W4 + b4)      # [512]

Distribution: nodes (and their in-edges) sharded contiguously across the 8
NeuronCores; the per-layer scaled feature table hs = (x @ W) * dinv is
all-gathered so every core can gather messages for its local destination
nodes; edge aggregation is a one-hot (selection-matrix) matmul accumulated in
PSUM per 128-destination tile; pooled sums/counts are all-reduced.

The edge gather uses dma_gather (SWDGE) against a bf16 row-padded feature
table (256B rows) in HBM, banked by 32768 rows to fit int16 indices.

Degree normalization (dinv = rsqrt(deg)) depends only on edge_index, so it is
precomputed on the host and uploaded; the per-node bias is folded into the
PSUM edge aggregation as a rank-1 bf16 matmul outer(sqrt(deg), b) so the
whole conv epilogue is one DVE add (self term) + one scalar-engine
Relu/Copy with per-partition dinv scale.
"""

import numpy as np

P = 128
DIM = 64


class CFG:
    def __init__(self, n=100000, e=1600000, g=512, cores=8, maxch_call=48):
        self.N = n
        self.E = e
        self.G = g
        self.R = cores
        self.PC = -(-n // cores)            # nodes per core (pre-pad)
        self.PC = -(-self.PC // P) * P      # pad to 128
        self.T = self.PC // P               # dst tiles per core
        self.NP = self.PC * cores           # padded global nodes
        self.NB = 4                         # banks = tile quarters
        # quarter boundaries in local tiles; bank k spans tiles [qb[k], qb[k+1])
        self.qb = [(self.T * q) // self.NB for q in range(self.NB + 1)]
        self.qrows = [(self.qb[k + 1] - self.qb[k]) * P
                      for k in range(self.NB)]       # per-core rows in bank k
        self.subrows = [r * cores for r in self.qrows]   # total rows in bank k
        assert max(self.subrows) <= 32767, self.subrows
        self.MAXCH = maxch_call             # max chunks per gather call
        self.skip_gather = False
        self.skip_edge_mm = False
        self.nqueues = 4
        self.skip_oh = False
        self.skip_ag = False
        self.skip_tail = False
        self.repeat = 1
        self.msg_bufs = 5
        self.zero_idx = False
        self.late_ag = False
        self.xt_bufs = 1
        self.oh_bufs = 4
        self.sp_all = False


FULL = CFG()


# ---------------------------------------------------- host preprocessing ---

def _prep(edge_index, batch, cfg):
    """Partition/pad edges; build per-core device arrays + shared schedule."""
    c = cfg
    src = np.asarray(edge_index[0], dtype=np.int64)
    dst = np.asarray(edge_index[1], dtype=np.int64)
    batch = np.asarray(batch, dtype=np.int64)

    core = dst // c.PC
    tloc = (dst % c.PC) // P
    slot = dst % P
    # bank = tile-quarter of the source node; row = core-major within bank
    qb = np.asarray(c.qb)
    qrows = np.asarray(c.qrows)
    csrc = src // c.PC
    lt = (src % c.PC) // P
    sslot = src % P
    bank = np.searchsorted(qb, lt, side="right") - 1
    ib = (csrc * qrows[bank] + (lt - qb[bank]) * P + sslot).astype(np.int64)

    key = ((core * c.T + tloc) * c.NB + bank)
    order = np.lexsort((ib, key))
    key_s = key[order]
    ib_s = ib[order].astype(np.int16)
    slot_s = slot[order].astype(np.float32)

    nkey = c.R * c.T * c.NB
    cnts = np.bincount(key_s, minlength=nkey)
    counts = cnts.reshape(c.R, c.T, c.NB)
    starts_flat = np.concatenate([[0], np.cumsum(cnts)])

    C_tb = -(-counts.max(axis=0) // P)          # [T, NB]
    for t in range(c.T):
        if C_tb[t].sum() == 0:
            C_tb[t, 0] = 1

    groups = []
    t0 = 0
    while t0 < c.T:
        t1 = t0
        while t1 < c.T:
            nch = C_tb[t0:t1 + 1].sum(axis=0).max()
            if nch > c.MAXCH and t1 > t0:
                break
            t1 += 1
        groups.append((t0, t1))
        t0 = t1

    chunk_of = np.zeros((c.T, c.NB), dtype=np.int64)
    calls = []       # (bank, t0, t1, chunk0, nch, idx_col0)
    CH = 0
    icol = 0
    for (t0, t1) in groups:
        for b in range(c.NB):
            ch0 = CH
            for t in range(t0, t1):
                chunk_of[t, b] = CH
                CH += int(C_tb[t, b])
            nch = CH - ch0
            if nch:
                calls.append((b, t0, t1, ch0, int(nch), icol))
                icol += nch * P // 16
    sched = dict(C_tb=C_tb, chunk_of=chunk_of, groups=groups, calls=calls,
                 CH=int(CH), ICOLS=int(icol))

    # degree (in-degree incl self-loop) -> dinv, sqrt(deg); host-only
    deg_all = np.bincount(dst, minlength=c.N).astype(np.float64) + 1.0
    dinv_all = (1.0 / np.sqrt(deg_all)).astype(np.float32)
    sdeg_all = np.sqrt(deg_all).astype(np.float32)
    # padded global dinv, tile-major [128, R*T] (identical on every core)
    dinvP = np.ones(c.NP, dtype=np.float32)
    for r in range(c.R):
        n0 = r * c.PC
        nr = max(0, min(c.N - n0, c.PC))
        if nr:
            dinvP[r * c.PC:r * c.PC + nr] = dinv_all[n0:n0 + nr]
    dinvG = dinvP.reshape(c.R * c.T, P).T.copy()          # [128, TG]

    per_core = []
    for r in range(c.R):
        idxw = np.zeros((P, icol), dtype=np.int16)
        dstl = np.full((P, CH), -1.0, dtype=np.float32)
        for (b, t0, t1, ch0, nch, col0) in calls:
            # pad slots gather spread-out (harmless) rows: a constant pad
            # row serializes on one HBM bank and stalls the whole queue
            li = ((np.arange(nch * P, dtype=np.int64) * 97 + b * 1009)
                  % c.subrows[b]).astype(np.int16)
            for t in range(t0, t1):
                k = (r * c.T + t) * c.NB + b
                s0, s1 = starts_flat[k], starts_flat[k + 1]
                n = int(s1 - s0)
                if n == 0:
                    continue
                o = int(chunk_of[t, b] - ch0) * P
                li[o:o + n] = ib_s[s0:s1]
                cpos = int(chunk_of[t, b])
                ii = np.arange(n)
                dstl[ii % P, cpos + ii // P] = slot_s[s0:s1]
            w = li.reshape(-1, 16).T                      # [16, ncol]
            idxw[:, col0:col0 + nch * P // 16] = np.tile(w, (8, 1))
        n0 = r * c.PC
        nreal = max(0, min(c.N - n0, c.PC))
        bat = np.full(c.PC, -1.0, dtype=np.float32)
        dv = np.ones(c.PC, dtype=np.float32)
        sd = np.ones(c.PC, dtype=np.float32)
        if nreal > 0:
            bat[:nreal] = batch[n0:n0 + nreal].astype(np.float32)
            dv[:nreal] = dinv_all[n0:n0 + nreal]
            sd[:nreal] = sdeg_all[n0:n0 + nreal]
        batchF = bat.reshape(c.T, P).T.copy()             # [128, T]
        dinvF = dv.reshape(c.T, P).T.copy()               # [128, T]
        sdegF = sd.reshape(1, c.PC).copy()                # [1, PC]
        if nreal > 0:
            g0 = int(batch[n0])
            ghi = int(batch[min(n0 + nreal, c.N) - 1])
            assert ghi - g0 < P, (r, g0, ghi)
        else:
            g0 = c.G - 1
        per_core.append(dict(idxw=idxw, dstl=dstl, batchF=batchF, g0=g0,
                             dinvF=dinvF, sdegF=sdegF, dinvG=dinvG))
    return sched, per_core


# ------------------------------------------------------- program builder ---

def build_program(cfg, sched):
    import concourse.bass as bass
    import concourse.bacc as bacc
    import concourse.mybir as mybir
    import concourse.tile as tile
    from concourse.tile import add_dep_helper

    c = cfg
    dt = mybir.dt
    f32 = dt.float32
    edt = dt.bfloat16
    ROWE = P                             # table row elements (256B rows)
    C_tb, chunk_of, calls = sched["C_tb"], sched["chunk_of"], sched["calls"]
    CH, ICOLS = sched["CH"], sched["ICOLS"]
    T, NB = c.T, c.NB
    ACT = mybir.ActivationFunctionType

    def tile_chunks(t):
        return [(b, k) for b in range(NB) for k in range(int(C_tb[t, b]))]

    def call_chunk0(t, b):
        for (bb, tt0, tt1, c0, nn, _c) in calls:
            if bb == b and tt0 <= t < tt1:
                return c0
        raise AssertionError((t, b))

    nc = bacc.Bacc("TRN2", target_bir_lowering=False, debug=False,
                   num_devices=c.R, num_swdge_queues=c.nqueues)

    TG = c.T * c.R                       # global padded tiles

    # ---- I/O ----
    xT_p = nc.declare_dram_parameter("xT", [DIM, c.PC], f32, isOutput=False)
    W_p = [nc.declare_dram_parameter(f"W{i+1}", [DIM, DIM if i < 3 else 1],
                                     f32, isOutput=False) for i in range(4)]
    b_p = [nc.declare_dram_parameter(f"b{i+1}", [1, DIM if i < 3 else 1],
                                     f32, isOutput=False) for i in range(4)]
    bbf_p = [nc.declare_dram_parameter(f"bb{i+1}", [1, DIM], edt,
                                       isOutput=False) for i in range(2)]
    iotaM_p = nc.declare_dram_parameter("iotaM", [P, P], f32, isOutput=False)
    id128_p = nc.declare_dram_parameter("id128", [P, P], f32, isOutput=False)
    ones1_p = nc.declare_dram_parameter("ones1", [1, P], f32, isOutput=False)
    idxw_p = nc.declare_dram_parameter("idxw", [P, ICOLS], dt.int16,
                                       isOutput=False)
    dstl_p = nc.declare_dram_parameter("dstl", [P, CH], f32, isOutput=False)
    batchF_p = nc.declare_dram_parameter("batchF", [P, T], f32, isOutput=False)
    dinvF_p = nc.declare_dram_parameter("dinvF", [P, T], f32, isOutput=False)
    sdegF_p = nc.declare_dram_parameter("sdegF", [1, c.PC], edt,
                                        isOutput=False)
    g0f_p = nc.declare_dram_parameter("g0f", [1, 1], f32, isOutput=False)
    g0i_p = nc.declare_dram_parameter("g0i", [1, 1], dt.uint32, isOutput=False)
    pred_p = nc.declare_dram_parameter("pred", [c.G, 1], f32, isOutput=True)

    # ---- internal DRAM (per repeat for clean timing isolation) ----
    GTOT = -(-(c.G + P) // P) * P
    hsl_a, tab1_a, tab2_a, pool_loc_a, pool_red_a = [], [], [], [], []
    for r_ in range(c.repeat):
        hsl_a.append([nc.dram_tensor(f"hs{i}_local_{r_}", [c.PC, ROWE], edt)
                      for i in (1, 2)])
        tab1_a.append([nc.dram_tensor(f"hs1_tab{k}_{r_}",
                                      [c.subrows[k], ROWE], edt,
                                      addr_space="Shared")
                       for k in range(c.NB)])
        tab2_a.append([nc.dram_tensor(f"hs2_tab{k}_{r_}",
                                      [c.subrows[k], ROWE], edt,
                                      addr_space="Shared")
                       for k in range(c.NB)])
        pool_loc_a.append(nc.dram_tensor(f"pool_local_{r_}",
                                         [GTOT, DIM + 1], f32))
        pool_red_a.append(nc.dram_tensor(f"pool_red_{r_}", [GTOT, DIM + 1],
                                         f32, addr_space="Shared"))

    rg = [list(range(c.R))]

    with tile.TileContext(nc) as tc:
        with (
            tc.tile_pool(name="const", bufs=1) as cpool,
            tc.tile_pool(name="msg", bufs=c.msg_bufs) as mpool,
            tc.tile_pool(name="oh", bufs=c.oh_bufs) as ohpool,
            tc.tile_pool(name="work", bufs=3) as wpool,
            tc.tile_pool(name="ps_agg", bufs=2, space="PSUM") as pagg,
            tc.tile_pool(name="ps_h", bufs=2, space="PSUM") as ph,
            tc.tile_pool(name="ps_tr", bufs=2, space="PSUM") as ptr,
            tc.tile_pool(name="ps_pool", bufs=1, space="PSUM") as ppool,
        ):

            def load(pool, ap, shape, dtype=f32, name=None):
                t = pool.tile(shape, dtype, tag=name)
                nc.sync.dma_start(out=t[:], in_=ap)
                return t

            iotaM = load(cpool, iotaM_p[:, :], [P, P], name="iotaM")
            id128 = load(cpool, id128_p[:, :], [P, P], name="id128")
            ones1 = load(cpool, ones1_p[:, :], [1, P], name="ones1")
            Ws = [load(cpool, W_p[i][:, :], [DIM, DIM if i < 3 else 1],
                       name=f"W{i}") for i in range(4)]
            bs = [load(cpool, b_p[i][:, :], [1, DIM if i < 3 else 1],
                       name=f"b{i}") for i in range(4)]
            bbf = [load(cpool, bbf_p[i][:, :], [1, DIM], edt, name=f"bbf{i}")
                   for i in range(2)]
            idxw = load(cpool, idxw_p[:, :], [P, ICOLS], dt.int16, name="idxw")
            batchF = load(cpool, batchF_p[:, :], [P, T], name="batchF")
            dinv = load(cpool, dinvF_p[:, :], [P, T], name="dinvF")
            sdeg = load(cpool, sdegF_p[:, :], [1, c.PC], edt, name="sdegF")
            g0f = load(cpool, g0f_p[:, :], [1, 1], name="g0f")

            iotaE = cpool.tile([P, P], edt, tag="iotaE")
            nc.vector.tensor_copy(out=iotaE[:], in_=iotaM[:])
            dstl = load(cpool, dstl_p[:, :], [P, CH], name="dstl")

            # bias broadcast mats via PE outer product: ones1.T @ b
            bB = []
            for i in range(2, 4):
                w = DIM if i < 3 else 1
                pb = ptr.tile([P, w], f32, tag="tr")
                nc.tensor.matmul(out=pb[:], lhsT=ones1[:], rhs=bs[i][:],
                                 start=True, stop=True)
                sb = cpool.tile([P, w], f32, tag=f"bB{i}")
                nc.scalar.copy(out=sb[:], in_=pb[:])
                bB.append(sb)
            pg = ptr.tile([P, 1], f32, tag="tr")
            nc.tensor.matmul(out=pg[:], lhsT=ones1[:], rhs=g0f[:],
                             start=True, stop=True)
            g0B = cpool.tile([P, 1], f32, tag="g0B")
            nc.scalar.copy(out=g0B[:], in_=pg[:])

            # persistent SBUF feature tables (f32, this core's nodes)
            hs_sb = [cpool.tile([P, T * DIM], f32, tag=f"hs_sb{i}",
                                name=f"hs_sb{i}") for i in range(2)]

            # rotating bf16 row-padded staging tiles (pad cols pre-zeroed)
            hsb_bufs = [cpool.tile([P, ROWE], edt, tag=f"hsb{i}",
                                   name=f"hsb{i}") for i in range(4)]
            for hb in hsb_bufs:
                nc.vector.memset(hb[:, DIM:ROWE], 0.0)

            # pool staging tiles with constant ones column
            pt_bufs = [cpool.tile([P, DIM + 1], edt, tag=f"pt{i}",
                                  name=f"pt{i}") for i in range(3)]
            for pb_ in pt_bufs:
                nc.vector.memset(pb_[:, DIM:DIM + 1], 1.0)

            oh_const = cpool.tile([P, P], edt, tag="ohc")
            nc.vector.memset(oh_const[:], 0.0)

            def onehot_for(chunk):
                if c.skip_oh:
                    return oh_const
                oh = ohpool.tile([P, P], edt, tag="oh")
                nc.vector.tensor_scalar(
                    out=oh[:], in0=iotaE[:],
                    scalar1=dstl[:, chunk:chunk + 1],
                    scalar2=None, op0=mybir.AluOpType.is_equal)
                return oh

            for _rep in range(c.repeat):
                hsl_r = hsl_a[_rep]
                tab1 = tab1_a[_rep]
                tab2 = tab2_a[_rep]
                pool_loc_r = pool_loc_a[_rep]
                pool_red_r = pool_red_a[_rep]

                ag_insts = [[None] * c.NB, [None] * c.NB]

                # ---------- hs writer: per-tile rows + quartered sub-AGs ----
                def emit_hs(t, hp, layer_i):
                    """hs = hp * dinv[t]; f32 in SBUF + bf16 row to HBM;
                    fire the sub-AllGather once a tile quarter completes."""
                    nc.scalar.mul(out=hs_sb[layer_i][:, t * DIM:(t + 1) * DIM],
                                  in_=hp[:], mul=dinv[:, t:t + 1])
                    hsb = hsb_bufs[t % len(hsb_bufs)]
                    nc.scalar.mul(out=hsb[:, 0:DIM], in_=hp[:],
                                  mul=dinv[:, t:t + 1])
                    nc.sync.dma_start(
                        out=hsl_r[layer_i][t * P:(t + 1) * P, :], in_=hsb[:])
                    tabs = tab1 if layer_i == 0 else tab2
                    for k in range(c.NB):
                        fire_at = T - 1 if c.late_ag else c.qb[k + 1] - 1
                        if t == fire_at and not c.skip_ag:
                            ag_insts[layer_i][k] = nc.gpsimd.collective_compute(
                                "AllGather", mybir.AluOpType.bypass,
                                replica_groups=rg,
                                ins=[hsl_r[layer_i][c.qb[k] * P:
                                                    c.qb[k + 1] * P, :]],
                                outs=[tabs[k][:, :]])

                # ---------- layer-1 features: h1 = x @ W1 ----------
                with tc.tile_pool(name="xTp", bufs=c.xt_bufs) as xpool:
                    for k in range(c.NB):
                        tq0, tq1 = c.qb[k], c.qb[k + 1]
                        xTq = xpool.tile([DIM, (tq1 - tq0) * P], f32,
                                         tag="xT", name="xTq")
                        nc.sync.dma_start(out=xTq[:],
                                          in_=xT_p[:, tq0 * P:tq1 * P])
                        for t in range(tq0, tq1):
                            hp = ph.tile([P, DIM], f32, tag="h")
                            nc.tensor.matmul(
                                out=hp[:],
                                lhsT=xTq[:, (t - tq0) * P:(t - tq0 + 1) * P],
                                rhs=Ws[0][:], start=True, stop=True)
                            emit_hs(t, hp, 0)

                # ---------- edge-aggregation layer ----------
                def run_layer(tabs, layer_i, bbf_t, ag_insts, emit_tile_out):
                    gat_of = {}
                    for ci, (b, t0, t1, ch0, nchv, col0) in enumerate(calls):
                        m = mpool.tile([P, nchv, ROWE], edt, tag="msg")
                        if not c.skip_gather:
                            g = nc.gpsimd.dma_gather(
                                m[:], tabs[b][0:c.subrows[b], :],
                                idxw[:, col0:col0 + nchv * P // 16], nchv * P,
                                nchv * P, ROWE,
                                single_packet=(c.sp_all or nchv * P <= 1024),
                                queue_num=ci % c.nqueues)
                            if ag_insts is not None and ag_insts[b] is not None:
                                add_dep_helper(g.ins, ag_insts[b].ins)
                        else:
                            nc.vector.memset(m[:, 0, :], 0.0)
                        for t in range(t0, t1):
                            gat_of[(t, b)] = m
                    for t in range(T):
                        chunks = tile_chunks(t)
                        if c.skip_edge_mm:
                            chunks = chunks[:1]
                        ap = pagg.tile([P, DIM], f32, tag="agg")
                        # rank-1 bias term: outer(sqrt(deg), b) in bf16
                        nc.tensor.matmul(
                            out=ap[:], lhsT=sdeg[:, t * P:(t + 1) * P],
                            rhs=bbf_t[:], start=True, stop=False)
                        for j, (b, k) in enumerate(chunks):
                            oh = onehot_for(int(chunk_of[t, b]) + k)
                            m = gat_of[(t, b)]
                            kk = int(chunk_of[t, b]) - call_chunk0(t, b) + k
                            nc.tensor.matmul(
                                out=ap[:], lhsT=oh[:], rhs=m[:, kk, 0:DIM],
                                start=False, stop=(j == len(chunks) - 1))
                        z = wpool.tile([P, DIM], f32, tag="z")
                        nc.vector.tensor_tensor(
                            out=z[:], in0=ap[:],
                            in1=hs_sb[layer_i][:, t * DIM:(t + 1) * DIM],
                            op=mybir.AluOpType.add)
                        emit_tile_out(t, z)

                # layer-1 tile epilogue: relu*dinv -> transpose -> h2 -> hs2
                def l1_out(t, z):
                    o1 = wpool.tile([P, DIM], f32, tag="o1")
                    nc.scalar.activation(out=o1[:], in_=z[:], func=ACT.Relu,
                                         scale=dinv[:, t:t + 1])
                    tp = ptr.tile([DIM, P], f32, tag="tr")
                    nc.tensor.transpose(out=tp[:], in_=o1[:], identity=id128[:])
                    o1T = wpool.tile([DIM, P], f32, tag="o1Ts")
                    nc.scalar.copy(out=o1T[:], in_=tp[:])
                    hp = ph.tile([P, DIM], f32, tag="h")
                    nc.tensor.matmul(out=hp[:], lhsT=o1T[:], rhs=Ws[1][:],
                                     start=True, stop=True)
                    emit_hs(t, hp, 1)

                run_layer(tab1, 0, bbf[0], ag_insts[0], l1_out)

                psum_pool = ppool.tile([P, DIM + 1], f32, tag="pool")

                def l2_out(t, z):
                    pt = pt_bufs[t % len(pt_bufs)]
                    nc.scalar.mul(out=pt[:, 0:DIM], in_=z[:],
                                  mul=dinv[:, t:t + 1])
                    og = ohpool.tile([P, P], edt, tag="ohg")
                    nc.vector.tensor_scalar(
                        out=og[:], in0=iotaM[:], scalar1=g0B[:, 0:1],
                        scalar2=batchF[:, t:t + 1], op0=mybir.AluOpType.add,
                        op1=mybir.AluOpType.is_equal)
                    nc.tensor.matmul(out=psum_pool[:], lhsT=og[:], rhs=pt[:],
                                     start=(t == 0), stop=(t == T - 1))

                run_layer(tab2, 1, bbf[1], ag_insts[1], l2_out)

                # ---------- pool finalize + AllReduce ----------
                poolsb = wpool.tile([P, DIM + 1], f32, tag="poolsb")
                nc.scalar.copy(out=poolsb[:], in_=psum_pool[:])
                if c.skip_tail:
                    nc.sync.dma_start(out=pred_p[0:P, :], in_=poolsb[:, 0:1])
                else:
                    zt = wpool.tile([P, DIM + 1], f32, tag="zt")
                    nc.vector.memset(zt[:], 0.0)
                    zdmas = []
                    for q in range(GTOT // P):
                        zdmas.append(nc.sync.dma_start(
                            out=pool_loc_r[q * P:(q + 1) * P, :], in_=zt[:]))
                    g0reg = nc.sync.alloc_register(f"g0reg{_rep}")
                    nc.sync.reg_load(g0reg, g0i_p[0:1, 0:1])
                    g0val = nc.sync.snap(g0reg, donate=True, min_val=0,
                                         max_val=GTOT - P)
                    wdma = nc.sync.dma_start(
                        out=pool_loc_r[bass.ds(g0val, P), :], in_=poolsb[:])
                    for zd in zdmas:
                        add_dep_helper(wdma.ins, zd.ins)
                    ar = nc.gpsimd.collective_compute(
                        "AllReduce", mybir.AluOpType.add, replica_groups=rg,
                        ins=[pool_loc_r[:, :]], outs=[pool_red_r[:, :]])
                    add_dep_helper(ar.ins, wdma.ins)

                    # ---------- head ----------
                    for q in range(-(-c.G // P)):
                        gq = min(P, c.G - q * P)
                        S = wpool.tile([P, DIM + 1], f32, tag="S")
                        d = nc.sync.dma_start(out=S[:],
                                              in_=pool_red_r[q * P:q * P + P, :])
                        add_dep_helper(d.ins, ar.ins)
                        cm = wpool.tile([P, 1], f32, tag="cm")
                        nc.vector.tensor_scalar(out=cm[:], in0=S[:, DIM:DIM + 1],
                                                scalar1=1.0, scalar2=None,
                                                op0=mybir.AluOpType.max)
                        ci = wpool.tile([P, 1], f32, tag="ci")
                        nc.vector.reciprocal(out=ci[:], in_=cm[:])
                        gt = wpool.tile([P, DIM], f32, tag="gt")
                        nc.vector.tensor_scalar(out=gt[:], in0=S[:, 0:DIM],
                                                scalar1=ci[:, 0:1], scalar2=None,
                                                op0=mybir.AluOpType.mult)
                        tp = ptr.tile([DIM, P], f32, tag="tr")
                        nc.tensor.transpose(out=tp[:], in_=gt[:], identity=id128[:])
                        gT = wpool.tile([DIM, P], f32, tag="gT")
                        nc.scalar.copy(out=gT[:], in_=tp[:])
                        zp = ph.tile([P, DIM], f32, tag="h")
                        nc.tensor.matmul(out=zp[:], lhsT=gT[:], rhs=Ws[2][:],
                                         start=True, stop=True)
                        zz = wpool.tile([P, DIM], f32, tag="zz")
                        nc.vector.tensor_tensor(out=zz[:], in0=zp[:], in1=bB[0][:],
                                                op=mybir.AluOpType.add)
                        nc.vector.tensor_scalar(out=zz[:], in0=zz[:], scalar1=0.0,
                                                scalar2=None, op0=mybir.AluOpType.max)
                        tp2 = ptr.tile([DIM, P], f32, tag="tr")
                        nc.tensor.transpose(out=tp2[:], in_=zz[:], identity=id128[:])
                        zT = wpool.tile([DIM, P], f32, tag="zT")
                        nc.scalar.copy(out=zT[:], in_=tp2[:])
                        pp = ptr.tile([P, 1], f32, tag="tr")
                        nc.tensor.matmul(out=pp[:], lhsT=zT[:], rhs=Ws[3][:],
                                         start=True, stop=True)
                        pr = wpool.tile([P, 1], f32, tag="pr")
                        nc.vector.tensor_tensor(out=pr[:], in0=pp[:], in1=bB[1][:],
                                                op=mybir.AluOpType.add)
                        nc.sync.dma_start(out=pred_p[q * P:q * P + gq, :],
                                          in_=pr[:gq, :])
    nc.compile()
    return nc


# --------------------------------------------------------------- runner ---

def _make_in_maps(x, W1, b1, W2, b2, W3, b3, W4, b4, cfg, per_core):
    import ml_dtypes
    c = cfg
    iotaM = np.tile(np.arange(P, dtype=np.float32)[None, :], (P, 1))
    id128 = np.eye(P, dtype=np.float32)
    ones1 = np.ones((1, P), dtype=np.float32)
    bf16 = ml_dtypes.bfloat16
    xf = np.zeros((c.NP, DIM), dtype=np.float32)
    for r in range(c.R):
        n0 = r * c.PC
        nreal = max(0, min(c.N - n0, c.PC))
        if nreal:
            xf[r * c.PC:r * c.PC + nreal] = np.asarray(
                x[n0:n0 + nreal], dtype=np.float32)
    xTf = np.ascontiguousarray(xf.T)
    maps = []
    for r in range(c.R):
        pc = per_core[r]
        maps.append({
            "xT": np.ascontiguousarray(xf[r * c.PC:(r + 1) * c.PC].T),
            "W1": np.asarray(W1, np.float32),
            "W2": np.asarray(W2, np.float32),
            "W3": np.asarray(W3, np.float32),
            "W4": np.asarray(W4, np.float32).reshape(DIM, 1),
            "b1": np.asarray(b1, np.float32).reshape(1, DIM),
            "b2": np.asarray(b2, np.float32).reshape(1, DIM),
            "b3": np.asarray(b3, np.float32).reshape(1, DIM),
            "b4": np.asarray(b4, np.float32).reshape(1, 1),
            "bb1": np.asarray(b1, np.float32).reshape(1, DIM).astype(bf16),
            "bb2": np.asarray(b2, np.float32).reshape(1, DIM).astype(bf16),
            "iotaM": iotaM, "id128": id128, "ones1": ones1,
            "idxw": (np.zeros_like(pc["idxw"]) if c.zero_idx
                     else pc["idxw"]),
            "dstl": pc["dstl"], "batchF": pc["batchF"],
            "dinvF": pc["dinvF"],
            "sdegF": pc["sdegF"].astype(bf16),
            "g0f": np.array([[float(pc["g0"])]], dtype=np.float32),
            "g0i": np.array([[pc["g0"]]], dtype=np.uint32),
        })
    return maps


def kernel(x, edge_index, batch, W1, b1, W2, b2, W3, b3, W4, b4,
           cfg=None, run=None):
    import sys
    if "/opt/trn_rl_repo" not in sys.path:
        sys.path.insert(0, "/opt/trn_rl_repo")
    cfg = cfg or FULL
    x = np.asarray(x)
    edge_index = np.asarray(edge_index)
    batch = np.asarray(batch)
    sched, per_core = _prep(edge_index, batch, cfg)
    nc = build_program(cfg, sched)
    maps = _make_in_maps(x, W1, b1, W2, b2, W3, b3, W4, b4, cfg, per_core)
    if run is not None:                 # custom runner (e.g. simulator)
        return run(nc, maps)
    from concourse.bass_utils import run_bass_kernel_spmd
    res = run_bass_kernel_spmd(nc, maps, list(range(cfg.R)))
    return np.asarray(res.results[0]["pred"]).reshape(-1).astype(np.float32)
